# revision 22
# baseline (speedup 1.0000x reference)
"""Trainium2 Bass kernel for nn_AppearanceLoss (keypoint patch CNN MSE).

Host: crops 33x33 patches at keypoint locations, builds full im2col
(27 rows = 3c x 3dy x 3dx per patch) so conv1 is a single-shot matmul,
shards 256 keypoints across 8 NeuronCores.
Device: conv1 = 64x64 PE-tiling, 4 concurrent tiles per group (2 row
groups x 2 col groups; K=55 = 2 patches x 27 im2col rows + shared ones
row for bias), single-shot N<=496 matmuls, no accumulation passes;
conv2 = offset-accumulated K=64 matmuls over patch pairs, two
concurrent 64-row tiles, N=450; GAP fused into the PSUM eviction via
accum_out (relu+bias+sum in one ACT/DVE op, unscaled - host divides by
225^2); linear on feature diffs; Square+accum partial sums.
Host: sums 8 per-core partials into the scalar MSE.
"""

import sys

sys.path.insert(0, "/opt/trn_rl_repo")

from collections import deque
from contextlib import ExitStack

import ml_dtypes
import numpy as np

import concourse.bass as bass  # noqa: F401
import concourse.tile as tile
from concourse import bacc, bass_utils, mybir

SIGMA = 16
PATCH = 33  # 2*SIGMA+1
HOUT = 31  # conv1 valid output: 33-3+1
COUT = 15  # conv2 stride-2 valid output: (31-3)//2+1
B, K, H = 4, 64, 256
NCORES = 8
NKP = B * K  # 256 keypoints total
KPC = NKP // NCORES  # 32 keypoints per core
NPATCH = KPC * B  # 128 patches per core per set
NQ = NPATCH // 4  # 32 quads per set
NQT = 2 * NQ  # 64 quads total per core
NG = NQT // 2  # 32 pair-groups (16 ground + 16 sat)
KIM = 55  # conv1 im2col rows per pair: 2*27 + ones
BF16 = mybir.dt.bfloat16
F32 = mybir.dt.float32
NPBF16 = ml_dtypes.bfloat16

_CACHE: dict = {}


def _build_graph():
    nc = bacc.Bacc(
        "TRN2",
        target_bir_lowering=False,
        debug=False,
        enable_asserts=False,
        num_devices=NCORES,
    )
    # conv1 im2col input: per group G, partition 64R + 27a + k holds
    # im2col row k (k = 9c+3dy+dx) of patch (G,R,cg,a); partition
    # 64R+54 = 1.0 (bias row); free dims [cg pair-sel, 31 y, 31 x].
    xi_d = nc.dram_tensor(
        "xi", [NG, 2, KIM, 2, HOUT, HOUT], BF16, kind="ExternalInput"
    ).ap()
    w1_d = nc.dram_tensor("w1", [128, 64], BF16, kind="ExternalInput").ap()
    w2_d = nc.dram_tensor("w2", [128, 9, 128], BF16, kind="ExternalInput").ap()
    b2_d = nc.dram_tensor("b2", [128, 1], F32, kind="ExternalInput").ap()
    wl_d = nc.dram_tensor("wl", [128, 128], BF16, kind="ExternalInput").ap()
    out_d = nc.dram_tensor("out", [128, 3], F32, kind="ExternalOutput").ap()

    RELU = mybir.ActivationFunctionType.Relu
    SQUARE = mybir.ActivationFunctionType.Square
    ADD = mybir.AluOpType.add
    MAX = mybir.AluOpType.max
    MULT = mybir.AluOpType.mult

    with ExitStack() as ctx:
        tc = ctx.enter_context(tile.TileContext(nc))
        const = ctx.enter_context(tc.tile_pool(name="const", bufs=1))
        xpool = ctx.enter_context(tc.tile_pool(name="x", bufs=6))
        hpool = ctx.enter_context(tc.tile_pool(name="h", bufs=6))
        gpool = ctx.enter_context(tc.tile_pool(name="g", bufs=1))
        spool = ctx.enter_context(tc.tile_pool(name="scr", bufs=4))
        pp1 = ctx.enter_context(tc.tile_pool(name="pp1", bufs=4, space="PSUM"))
        pp2 = ctx.enter_context(tc.tile_pool(name="pp2", bufs=4, space="PSUM"))

        # --- input prefetch first so the first conv1 group is ready
        # the moment warmup ends (input DMAs queue ahead of consts) ---
        xi_tiles: dict = {}

        def issue_dma(G):
            xt = xpool.tile([128, 2, HOUT, HOUT], BF16, tag="xi", name=f"xi_{G}")
            for R in range(2):
                nc.sync.dma_start(xt[64 * R : 64 * R + KIM], xi_d[G, R])
            xi_tiles[G] = xt

        issue_dma(0)
        w1_t = const.tile([128, 64], BF16)
        nc.sync.dma_start(w1_t[:], w1_d)
        issue_dma(1)
        # flat per-offset copies of w2 (2D contiguous tiles for fast LDW)
        w2os = []
        for o in range(9):
            w2o = const.tile([128, 128], BF16, name=f"w2o_{o}")
            nc.sync.dma_start(w2o[:], w2_d[:, o, :])
            w2os.append(w2o)
        issue_dma(2)
        b2_t = const.tile([128, 1], F32)
        nc.sync.dma_start(b2_t[:], b2_d)
        wl_t = const.tile([128, 128], BF16)
        nc.sync.dma_start(wl_t[:], wl_d)
        issue_dma(3)

        # gap col 4G+2q+jj (unscaled sum, host divides by 225^2);
        # partition 64a+m = patch (G, q, 2*jj+a) channel m
        gap = gpool.tile([128, NQT * 2], F32)
        res = gpool.tile([128, 3], F32)

        # PE warm-up burst spans the ~3.4us HAM window so the clock gate
        # is at 8/8 when real work starts; reads never-written SBUF so it
        # has no DMA dependency; Square+accum sink keeps it DCE-live
        junk = const.tile([128, 512], BF16, name="junk")
        nc.gpsimd.memset(junk[:], 0.5)
        wps = pp1.tile([128, 512], F32, tag="ps1", name="warm_ps")
        for i in range(8):
            nc.tensor.matmul(
                wps[:],
                junk[:, 0:128],
                junk[:],
                start=(i == 0),
                stop=(i == 7),
            )
        wscr = spool.tile([128, 512], F32, tag="wscr")
        nc.scalar.activation(wscr[:], wps[:], SQUARE, accum_out=res[:, 2:3])

        # greedy ACT/DVE load balancing: pick the engine with the least
        # accumulated estimated busy-ns for each PSUM eviction
        eng_ns = {"act": 0.0, "dve": 0.0}

        def pick_engine(act_cost, dve_cost):
            if eng_ns["act"] + act_cost <= eng_ns["dve"] + dve_cost:
                eng_ns["act"] += act_cost
                return "act"
            eng_ns["dve"] += dve_cost
            return "dve"

        def evict_relu(dst, src):
            # conv1 eviction: relu, f32 PSUM -> bf16 SBUF
            if pick_engine(550.0, 594.0) == "act":
                nc.scalar.activation(dst, src, RELU)
            else:
                nc.vector.tensor_scalar_max(dst, src, 0.0)

        def evict_gap(src, col):
            # conv2 fused eviction: gap[col] = sum_pos relu(x + b2)
            if pick_engine(580.0, 660.0) == "act":
                nc.scalar.activation(
                    src,
                    src,
                    RELU,
                    bias=b2_t[:],
                    accum_out=gap[:, col : col + 1],
                )
            else:
                scr = spool.tile([128, COUT * COUT], F32, tag="scr2", name="scr2")
                nc.vector.tensor_scalar(
                    scr[:], src, b2_t[:], 0.0, op0=ADD, op1=MAX
                )
                nc.vector.tensor_reduce(
                    gap[:, col : col + 1],
                    scr[:],
                    axis=mybir.AxisListType.X,
                    op=ADD,
                )

        def emit_conv1(G):
            # conv1: single-shot im2col matmuls, 4 concurrent 64x64 PE
            # tiles (2 row groups R x 2 col groups cg); K=55, M=64
            xt = xi_tiles.pop(G)
            h1 = hpool.tile([128, 2, HOUT, HOUT], BF16, tag="h1", name=f"h1_{G}")
            for y0, nr in ((0, 16), (16, 15)):
                pss = []
                for R in range(2):
                    ps = pp1.tile([128, nr, HOUT], F32, tag="ps1", name=f"c1_{R}")
                    for cg in range(2):
                        nc.tensor.matmul(
                            ps[64 * cg : 64 * cg + 64, :, :],
                            w1_t[64 * R : 64 * R + KIM, :],
                            xt[64 * R : 64 * R + KIM, cg, y0 : y0 + nr, :],
                            start=True,
                            stop=True,
                            tile_position=(64 * R, 64 * cg),
                        )
                    pss.append(ps)
                for R in range(2):
                    evict_relu(h1[:, R, y0 : y0 + nr, :], pss[R][:, :, :])
            return h1

        def emit_conv2(G, h1):
            # conv2 on the quad pair; pairs in concurrent row-tiles,
            # offset-outer/pair-inner for strict issue alternation
            ps2s = [
                pp2.tile(
                    [128, 2, COUT * COUT], F32, tag="ps2", name=f"ps2_{jj}"
                )
                for jj in range(2)
            ]
            # 4x (64,64) col-split tiles: 64-col LDWs (53ns vs ~95ns for
            # 128-col no-FWL loads). Both cg accumulation chains share a
            # bank at disjoint partitions: only cg0's o=0 uses start=True
            # (the bank-wide has_written clear); cg1's o=0 relies on the
            # cleared bits to overwrite-then-set (start=False).
            for o in range(9):
                dy, dx = o // 3, o % 3
                for jj in range(2):
                    p0 = 64 * jj
                    for cg in range(2):
                        nc.tensor.matmul(
                            ps2s[jj][64 * cg : 64 * cg + 64, :, :],
                            w2os[o][p0 : p0 + 64, 64 * cg : 64 * cg + 64],
                            h1[
                                p0 : p0 + 64,
                                :,
                                dy : dy + 29 : 2,
                                dx : dx + 29 : 2,
                            ],
                            start=(o == 0 and cg == 0),
                            stop=(o == 8),
                            skip_group_check=(cg == 1),
                            tile_position=(p0, 64 * cg),
                        )
            # fused eviction: relu(x + b2) summed over the 225 positions
            # straight into the gap column (in-place PSUM write for the
            # main output; accum_out carries the GAP sum)
            for jj in range(2):
                for q in range(2):
                    evict_gap(ps2s[jj][:, q, :], 4 * G + 2 * q + jj)

        # software-pipelined emission: conv1 runs two groups ahead of
        # conv2 so h1 evictions complete well before conv2 reads them
        dgb = gpool.tile([128, NQ * 2], BF16)
        pending: deque = deque()
        for G in range(NG):
            if G + 4 < NG:
                issue_dma(G + 4)
            h1 = emit_conv1(G)
            pending.append((G, h1))
            if len(pending) > 2:
                emit_conv2(*pending.popleft())
        while pending:
            emit_conv2(*pending.popleft())

        # linear on feature diffs (linear bias cancels), squared sums
        dg = spool.tile([128, NQ * 2], F32, tag="dg")
        nc.vector.tensor_sub(dg[:], gap[:, 0 : NQ * 2], gap[:, NQ * 2 : NQT * 2])
        nc.vector.tensor_copy(dgb[:], dg[:])
        for jj in range(2):
            p0 = 64 * jj
            ps3 = pp2.tile([128, NQ * 2], F32, tag="ps2", name=f"ps3_{jj}")
            nc.tensor.matmul(
                ps3[:],
                wl_t[p0 : p0 + 64, :],
                dgb[p0 : p0 + 64, :],
                start=True,
                stop=True,
                tile_position=(p0, 0),
            )
            scr3 = spool.tile([128, NQ * 2], F32, tag="scr3", name=f"scr3_{jj}")
            nc.scalar.activation(
                scr3[:], ps3[:], SQUARE, accum_out=res[:, jj : jj + 1]
            )
        nc.sync.dma_start(out_d, res[:])

    nc.compile()
    return nc


def _prep_weights(w1, b1, w2, b2, wl):
    # conv1 im2col weights: [64R + 27a + (9c+3dy+3?dx), 32a+m]
    w1i = np.zeros((128, 64), np.float32)
    for a in range(2):
        for c in range(3):
            for dy in range(3):
                for dx in range(3):
                    w1i[27 * a + 9 * c + 3 * dy + dx, 32 * a : 32 * a + 32] = w1[
                        :, c, dy, dx
                    ]
        w1i[54, 32 * a : 32 * a + 32] = b1
    w1i[64:119] = w1i[0:55]

    w2blk = np.zeros((128, 9, 128), np.float32)
    for jj in range(2):
        for j in range(2):
            for c in range(32):
                for o in range(9):
                    dy, dx = o // 3, o % 3
                    w2blk[64 * jj + 32 * j + c, o, 64 * j : 64 * j + 64] = w2[
                        :, c, dy, dx
                    ]
    b2q = np.tile(b2, 2)[:, None].astype(np.float32)  # unscaled
    wlrep = np.zeros((128, 128), np.float32)
    wlrep[0:64] = wl.T
    wlrep[64:128] = wl.T
    return (
        w1i.astype(NPBF16),
        w2blk.astype(NPBF16),
        np.ascontiguousarray(b2q),
        wlrep.astype(NPBF16),
    )


def _crop_all(images, kps):
    # images [B,3,H,W] f32; kps [NKP,2] normalized -> patches [NKP,B,3,P,P]
    hw = images.shape[-1]
    px = kps.astype(np.float32) * np.float32(hw)
    starts = np.clip(np.floor(px).astype(np.int32) - SIGMA, 0, hw - PATCH)
    out = np.empty((kps.shape[0], images.shape[0], 3, PATCH, PATCH), np.float32)
    for n in range(kps.shape[0]):
        x, y = int(starts[n, 0]), int(starts[n, 1])
        out[n] = images[:, :, y : y + PATCH, x : x + PATCH]
    return out


def _im2col_groups(pat):
    # pat [128,3,33,33] (one set for one core) -> [16, 2, 55, 2, 31, 31]
    # group g, row-half R, im2col row (27a+9c+3dy+dx | 54=ones), pair cg
    sw = np.lib.stride_tricks.sliding_window_view(pat, (HOUT, HOUT), axis=(2, 3))
    # sw[n, c, dy, dx, y, x] = pat[n, c, dy+y, dx+x]
    sw = sw.reshape(16, 2, 2, 2, 27, HOUT, HOUT)  # (g, R, cg, a, k, y, x)
    out = np.empty((16, 2, KIM, 2, HOUT, HOUT), np.float32)
    out[:, :, :54] = sw.transpose(0, 1, 3, 4, 2, 5, 6).reshape(
        16, 2, 54, 2, HOUT, HOUT
    )
    out[:, :, 54] = 1.0
    return out


def _make_in_maps(np_inputs):
    images_ground = np.asarray(np_inputs["images_ground"], np.float32)
    images_satellite = np.asarray(np_inputs["images_satellite"], np.float32)
    kg = np.asarray(np_inputs["keypoints_ground"], np.float32).reshape(-1, 2)
    ks = np.asarray(np_inputs["keypoints_satellite"], np.float32).reshape(-1, 2)
    w1 = np.asarray(np_inputs["w1"], np.float32)
    b1 = np.asarray(np_inputs["b1"], np.float32)
    w2 = np.asarray(np_inputs["w2"], np.float32)
    b2 = np.asarray(np_inputs["b2"], np.float32)
    wl = np.asarray(np_inputs["wl"], np.float32)

    pg = _crop_all(images_ground, kg)  # [256,4,3,33,33]
    ps = _crop_all(images_satellite, ks)
    w1i, w2blk, b2q, wlrep = _prep_weights(w1, b1, w2, b2, wl)

    in_maps = []
    for i in range(NCORES):
        sl = slice(i * KPC, (i + 1) * KPC)
        patg = pg[sl].reshape(NPATCH, 3, PATCH, PATCH)
        pats = ps[sl].reshape(NPATCH, 3, PATCH, PATCH)
        xi = np.concatenate(
            [_im2col_groups(patg), _im2col_groups(pats)], axis=0
        ).astype(NPBF16)
        in_maps.append(dict(xi=xi, w1=w1i, w2=w2blk, b2=b2q, wl=wlrep))
    return in_maps


def kernel(**inputs):
    in_maps = _make_in_maps(inputs)

    if "nc" not in _CACHE:
        _CACHE["nc"] = _build_graph()
    nc = _CACHE["nc"]

    results = bass_utils.run_bass_kernel_spmd(
        nc, in_maps, core_ids=list(range(NCORES))
    )
    total = np.float64(0.0)
    for r in results.results:
        total += np.asarray(r["out"], np.float64)[:, :2].sum()
    mse = total / (NKP * B * 128 * (COUT * COUT) ** 2)
    return np.asarray(mse, np.float32)


if __name__ == "__main__":
    rng = np.random.default_rng(0)
    ins = dict(
        images_ground=rng.standard_normal((B, 3, H, H)).astype(np.float32),
        images_satellite=rng.standard_normal((B, 3, H, H)).astype(np.float32),
        keypoints_ground=(0.2 + 0.6 * rng.random((B, K, 2))).astype(np.float32),
        keypoints_satellite=(0.2 + 0.6 * rng.random((B, K, 2))).astype(np.float32),
        w1=(rng.standard_normal((32, 3, 3, 3)) * 0.1).astype(np.float32),
        b1=np.zeros(32, np.float32),
        w2=(rng.standard_normal((64, 32, 3, 3)) * 0.05).astype(np.float32),
        b2=np.zeros(64, np.float32),
        wl=(rng.standard_normal((128, 64)) * 0.1).astype(np.float32),
        bl=np.zeros(128, np.float32),
        num_samples=K,
    )
    print("kernel out:", kernel(**ins))
